# revision 40
# baseline (speedup 1.0000x reference)
"""Causal self-attention on 8 Trainium2 NeuronCores.

Sharding: core c = 2*b + g handles batch b (of 4) and head-group g (of 2,
8 heads each). Per core: local qkv projection (bf16 matmuls), causal
flash-style attention in transposed-score layout (S^T = K @ Q^T so the
PV matmul needs no transposes; softmax denominator via a ones-column
appended to V; no max-subtraction -- scores are ~N(0,1)), then a PARTIAL
output projection against this head-group's rows of W_out^T. No
cross-core collectives: the host sums the two partial y's per batch
(the unshard step), so every core's NEFF runs fully independently.
"""

import os
import sys

import numpy as np

sys.path.insert(0, "/opt/trn_rl_repo")

import concourse.bass as bass  # noqa: E402
import concourse.mybir as mybir  # noqa: E402
import concourse.tile as tile  # noqa: E402
from concourse.bass_utils import run_bass_kernel_spmd  # noqa: E402
from concourse.vector_clock import ScopedClock  # noqa: E402

B, T, D = 4, 2048, 1024
H, HD = 16, 64
HL = H // 2          # heads per core
HDL = HL * HD        # 512 local head dims
NCB = D // 128       # 8 contraction blocks
NTB = T // 128       # 16 t blocks
TC = 512             # moving-dim chunk (one matmul must fit one PSUM bank)
NTC = T // TC
BF = mybir.dt.bfloat16
F32 = mybir.dt.float32

# ---------------------------------------------------------------------------
# Workaround: this walrus build rejects any instruction carrying more than
# one sync-wait ("Too many sync wait commands"). Split extra waits onto
# no-op carrier instructions on the same engine; same for the TileContext
# tail drain, which aggregates one wait per DMA queue.
_orig_commit = tile.TileContext._commit_instruction


def _split_waits(self, inst):
    si = inst.sync_info
    if si is None or len(si.on_wait) <= 1:
        return
    if inst.engine == mybir.EngineType.Unassigned:
        return
    waits = list(si.on_wait)
    for w in waits[:-1]:
        carrier = mybir.InstNoOp(
            name=self.nc.get_next_instruction_name(),
            sync_info=mybir.SyncInfo(on_wait=[w], on_update=[]),
            bass_nofuse=True,
            engine=inst.engine,
        )
        _orig_commit(self, carrier)
    try:
        si.on_wait = waits[-1:]
    except Exception:
        inst.sync_info = mybir.SyncInfo(
            on_wait=waits[-1:], on_update=list(si.on_update)
        )


def _patched_commit(self, inst, lazy_reg_writes=True):
    _split_waits(self, inst)
    return _orig_commit(self, inst, lazy_reg_writes)


def _patched_drain_and_barrier(self, tick_clock, wait_clock):
    drain_inst = self.nc.sync.drain()
    wait_clock.add_sem_waits(
        drain_inst.ins, ScopedClock({None: tick_clock.global_clock})
    )
    ins = drain_inst.ins
    si = ins.sync_info
    if si is not None and len(si.on_wait) > 1:
        waits = list(si.on_wait)
        try:
            si.on_wait = waits[:1]
        except Exception:
            ins.sync_info = mybir.SyncInfo(
                on_update=list(si.on_update), on_wait=waits[:1]
            )
        for w in waits[1:]:
            extra = self.nc.sync.drain()
            extra.ins.sync_info = mybir.SyncInfo(on_update=[], on_wait=[w])
    self.nc.all_engine_barrier()
    assert self.sems is not None
    popped = self.nc._tile_sem_poison_stack.pop()
    assert popped is self._sem_poison
    self.nc.clear_and_free_semaphores(list(self.sems.allocated().values()))
    self.nc.all_engine_barrier()


tile.TileContext._commit_instruction = _patched_commit
tile.TileContext._drain_and_barrier = _patched_drain_and_barrier
# ---------------------------------------------------------------------------

_PROG = None


def _build():
    nc = bass.Bass()
    xT_p = nc.declare_dram_parameter("xT", [D, T], BF, False)
    wq_p = nc.declare_dram_parameter("wqT", [D, HDL], BF, False)
    wk_p = nc.declare_dram_parameter("wkT", [D, HDL], BF, False)
    wv_p = nc.declare_dram_parameter("wvT", [D, HDL], BF, False)
    wo_p = nc.declare_dram_parameter("woT", [HDL, D], BF, False)
    mk_p = nc.declare_dram_parameter("mask", [128, 128], BF, False)
    yT_p = nc.declare_dram_parameter("yT", [D, T], BF, True)

    Exp = mybir.ActivationFunctionType.Exp
    MUL = mybir.AluOpType.mult
    TB2 = 1024

    with tile.TileContext(nc) as tc:
        with tc.tile_pool(name="persist", bufs=1) as pp:
            QT = pp.tile([128, HL // 2, T], BF)
            KT = pp.tile([128, HL // 2, T], BF)
            VB = pp.tile([128, NTB, HL, HD + 1], BF)
            OTo = pp.tile([128, HL // 2, T], BF)

            with (
                tc.tile_pool(name="ain", bufs=1) as pin,
                tc.tile_pool(name="se", bufs=3) as pse,
                tc.tile_pool(name="yout", bufs=3) as pyo,
                tc.tile_pool(name="ps_s", bufs=2, space="PSUM") as pss,
                tc.tile_pool(name="ps_o", bufs=2, space="PSUM") as pso,
            ):
                # first head pair's weight slices before the bulk x transfer
                # so the first projection matmul starts ~1.5us in; x streams
                # T-chunk-major underneath the projection compute
                WQ = pin.tile([128, NCB, HDL], BF)
                wq_r = wq_p.rearrange("(o p) d -> p o d", p=128)
                WK = pin.tile([128, NCB, HDL], BF)
                wk_r = wk_p.rearrange("(o p) d -> p o d", p=128)
                nc.sync.dma_start(WQ[:, :, 0:128], wq_r[:, :, 0:128])
                XT = pin.tile([128, NCB, T], BF)
                xT_r = xT_p.rearrange("(o p) t -> p o t", p=128)
                tsl0 = slice(0, TC)
                for cb in range(NCB):
                    nc.sync.dma_start(XT[:, cb, tsl0], xT_r[:, cb, tsl0])
                nc.sync.dma_start(WK[:, :, 0:128], wk_r[:, :, 0:128])
                WV = pin.tile([128, NCB, HDL], BF)
                nc.sync.dma_start(WV[:], wv_p.rearrange("(o p) d -> p o d", p=128))
                for tcc in range(1, NTC):
                    tsl = slice(tcc * TC, (tcc + 1) * TC)
                    for cb in range(NCB):
                        nc.sync.dma_start(XT[:, cb, tsl], xT_r[:, cb, tsl])
                nc.sync.dma_start(WQ[:, :, 128:HDL], wq_r[:, :, 128:HDL])
                nc.sync.dma_start(WK[:, :, 128:HDL], wk_r[:, :, 128:HDL])
                WO = pin.tile([128, HDL // 128, D], BF)
                nc.sync.dma_start(WO[:], wo_p.rearrange("(o p) d -> p o d", p=128))
                MK = pin.tile([128, 128], BF)
                nc.sync.dma_start(MK[:], mk_p[:])
                ONES = pin.tile([1, 64], BF)
                nc.vector.memset(ONES[:], 1.0)
                # only the denominator ones-column; [..., 0:64] is overwritten
                # by the V projection
                nc.vector.memset(VB[:, :, :, 64:65], 1.0)

                # deferred softmax epilogue: emitted one iteration late so the
                # PE reaches the denominator-broadcast matmul only after the
                # 1-lane dN copy has long finished (no PE stall)
                pending = [None]

                def emit_epilogue():
                    if pending[0] is None:
                        return
                    po, ib, tcc, pout = pending[0]
                    pending[0] = None
                    dN = pse.tile([1, TB2], BF, tag="rcpb")
                    nc.vector.tensor_copy(dN[:], pout[64:65, :])
                    prb = pss.tile([64, TB2], F32, tag="ps")
                    for half in range(2):
                        h0, h1 = half * 512, (half + 1) * 512
                        nc.tensor.matmul(
                            prb[:, h0:h1], ONES[:], dN[:, h0:h1],
                            start=True, stop=True,
                        )
                    rbs = pse.tile([64, TB2], F32, tag="rbs")
                    nc.vector.reciprocal(rbs[:], prb[:])
                    nc.vector.tensor_tensor(
                        OTo[po:po + 64, ib, tcc * TB2:(tcc + 1) * TB2],
                        pout[0:64, :],
                        rbs[:],
                        MUL,
                    )

                def proj_qk(ib):
                    for tcc in range(NTC):
                        tsl = slice(tcc * TC, (tcc + 1) * TC)
                        pq = pss.tile([128, TC], F32, tag="ps")
                        for cb in range(NCB):
                            nc.tensor.matmul(
                                pq[:],
                                WQ[:, cb, ib * 128:(ib + 1) * 128],
                                XT[:, cb, tsl],
                                start=(cb == 0),
                                stop=(cb == NCB - 1),
                            )
                        nc.vector.tensor_copy(QT[:, ib, tsl], pq[:])
                        pk = pss.tile([128, TC], F32, tag="ps")
                        for cb in range(NCB):
                            nc.tensor.matmul(
                                pk[:],
                                WK[:, cb, ib * 128:(ib + 1) * 128],
                                XT[:, cb, tsl],
                                start=(cb == 0),
                                stop=(cb == NCB - 1),
                            )
                        nc.vector.tensor_copy(KT[:, ib, tsl], pk[:])

                # ---- feeder: independent PE work units interleaved into the
                # ACT-bound attention stream so the in-order PE never idles.
                # Each step is a small closure (one chunk: psum alloc + 8
                # accumulation matmuls + evacuation copy). Drain points keep
                # the data dependencies: V chunks tb>=8 before any tcc=1
                # attention, proj(ib) before head pair ib.
                yT_r = yT_p.rearrange("(o p) t -> p o t", p=128)
                steps = []
                marks = {}

                def _v_chunk(tb):
                    def emit():
                        pv = pss.tile([128, HDL], F32, tag="ps", name="pv")
                        for cb in range(NCB):
                            nc.tensor.matmul(
                                pv[:],
                                XT[:, cb, tb * 128:(tb + 1) * 128],
                                WV[:, cb, :],
                                start=(cb == 0),
                                stop=(cb == NCB - 1),
                            )
                        nc.vector.tensor_copy(
                            VB[:, tb, :, 0:HD],
                            pv.rearrange("p (h e) -> p h e", h=HL),
                        )
                    return emit

                def _proj_chunk(ib, tcc, which):
                    def emit():
                        tsl = slice(tcc * TC, (tcc + 1) * TC)
                        W, OUT = (WQ, QT) if which == "q" else (WK, KT)
                        pt = pss.tile([128, TC], F32, tag="ps", name="pt")
                        for cb in range(NCB):
                            nc.tensor.matmul(
                                pt[:],
                                W[:, cb, ib * 128:(ib + 1) * 128],
                                XT[:, cb, tsl],
                                start=(cb == 0),
                                stop=(cb == NCB - 1),
                            )
                        nc.vector.tensor_copy(OUT[:, ib, tsl], pt[:])
                    return emit

                def _py_chunk(tcc, jb):
                    def emit():
                        tsl = slice(tcc * TC, (tcc + 1) * TC)
                        py = pss.tile([128, TC], F32, tag="ps", name="py")
                        for cb in range(HDL // 128):
                            nc.tensor.matmul(
                                py[:],
                                WO[:, cb, jb * 128:(jb + 1) * 128],
                                OTo[:, cb, tsl],
                                start=(cb == 0),
                                stop=(cb == HDL // 128 - 1),
                            )
                        yo = pyo.tile([128, TC], BF, tag="yo")
                        nc.vector.tensor_copy(yo[:], py[:])
                        nc.sync.dma_start(yT_r[:, jb, tsl], yo[:])
                    return emit

                for tb in range(NTB // 2, NTB):
                    steps.append(_v_chunk(tb))
                marks["v"] = len(steps)
                for ib in range(1, HL // 2):
                    for tcc in range(NTC):
                        steps.append(_proj_chunk(ib, tcc, "q"))
                        steps.append(_proj_chunk(ib, tcc, "k"))
                    marks[f"proj{ib}"] = len(steps)
                marks["attn_ok"] = len(steps)
                # first half of the out projection: depends on the tcc=0
                # epilogues of all heads, feedable during the last head's
                # tcc=1 attention
                for tcc in range(NTC // 2):
                    for jb in range(D // 128):
                        steps.append(_py_chunk(tcc, jb))

                fed = [0]

                def feed(n, limit=None):
                    lim = len(steps) if limit is None else marks[limit]
                    while n > 0 and fed[0] < lim:
                        steps[fed[0]]()
                        fed[0] += 1
                        n -= 1

                def drain(limit):
                    feed(len(steps), limit)

                kbctr = [0]

                def attn_head(h):
                    po = (h % 2) * 64
                    ib = h // 2
                    for tcc in range(T // TB2):
                        if h == 0 and tcc == 1:
                            drain("v")
                        kbmax = (tcc + 1) * TB2 // 128
                        pout = pso.tile([65, TB2], F32, tag="pout")

                        def emit_pv(kb, se, qs):
                            for half in range(2):
                                h0, h1 = half * 512, (half + 1) * 512
                                if qs >= h1:
                                    continue
                                lo = max(qs, h0)
                                nxt_qs = max(0, (kb + 1) * 128 - tcc * TB2)
                                nc.tensor.matmul(
                                    pout[:, lo:h1],
                                    VB[:, kb, h, :],
                                    se[:, lo:h1],
                                    start=(kb == 0),
                                    stop=(kb == kbmax - 1 or nxt_qs >= h1),
                                )

                        # lag-1 software pipeline: QK(kb+1) is emitted before
                        # PV(kb) so the in-order PE covers exp(kb)'s latency
                        prev = None
                        for kb in range(kbmax):
                            qs = max(0, kb * 128 - tcc * TB2)
                            ps_ = pss.tile([128, TB2], F32, tag="ps")
                            for half in range(2):
                                h0, h1 = half * 512, (half + 1) * 512
                                if qs >= h1:
                                    continue
                                lo = max(qs, h0)
                                nc.tensor.matmul(
                                    ps_[:, lo:h1],
                                    KT[po:po + 64, ib, kb * 128:(kb + 1) * 128],
                                    QT[po:po + 64, ib, tcc * TB2 + lo:tcc * TB2 + h1],
                                    start=True,
                                    stop=True,
                                )
                            se = pse.tile([128, TB2], BF, tag="se")
                            nc.scalar.activation(
                                se[:, qs:], ps_[:, qs:], Exp, scale=0.125
                            )
                            if kb * 128 >= tcc * TB2:
                                # on the (otherwise idle) Pool engine: keeps
                                # the exp->mask->PV chain off the DVE queue
                                nc.gpsimd.tensor_tensor(
                                    se[:, qs:qs + 128],
                                    se[:, qs:qs + 128],
                                    MK[:],
                                    MUL,
                                )
                            # filler goes BEFORE the PV that waits on exp(kb):
                            # the in-order PE chews it while ACT catches up.
                            # epilogue deferred to kb==4 so the PE has enough
                            # queued work to cover the 1.2us dN copy latency
                            if kb == 4:
                                emit_epilogue()
                            kbctr[0] += 1
                            if kbctr[0] % 2 == 0:
                                py_ok = h == H // 2 - 1 and tcc == 1 and kb >= 5
                                feed(1, None if py_ok else "attn_ok")
                            if prev is not None:
                                emit_pv(*prev)
                            prev = (kb, se, qs)
                        emit_pv(*prev)
                        pending[0] = (po, ib, tcc, pout)

                # upfront: first head pair's q/k projection and the first half
                # of the V projection (head 0 tcc=0 needs key blocks 0..7);
                # everything else is fed into the attention stream
                proj_qk(0)
                for tb in range(NTB // 2):
                    _v_chunk(tb)()

                for ib in range(HL // 2):
                    if ib > 0:
                        drain(f"proj{ib}")
                    attn_head(2 * ib)
                    attn_head(2 * ib + 1)

                # ---------- partial output projection (no collectives) -----
                # yT_partial[i, t] = sum_a W_out[i, g*512 + a] * attnoutT[a, t]
                # with a = 128*cb + p exactly matching OTo[p, cb, t]. The
                # first half was fed during the last head's attention; the
                # rest is the tail.
                emit_epilogue()
                feed(len(steps))
                for tcc in range(NTC // 2, NTC):
                    for jb in range(D // 128):
                        _py_chunk(tcc, jb)()

    return nc


last_results = None


def kernel(x, W_qkv, W_out):
    global _PROG, last_results
    import ml_dtypes

    bfq = ml_dtypes.bfloat16
    if _PROG is None:
        _PROG = _build()

    x = np.asarray(x, np.float32)
    W_qkv = np.asarray(W_qkv, np.float32)
    W_out = np.asarray(W_out, np.float32)
    mask = np.triu(np.ones((128, 128), np.float32)).astype(bfq)
    woT_full = np.ascontiguousarray(W_out.T)

    in_maps = []
    for c in range(8):
        b, g = c // 2, c % 2
        sl = slice(g * HDL, (g + 1) * HDL)
        in_maps.append(
            {
                "xT": np.ascontiguousarray(x[b].T).astype(bfq),
                "wqT": np.ascontiguousarray(W_qkv[sl].T).astype(bfq),
                "wkT": np.ascontiguousarray(W_qkv[D + g * HDL:D + (g + 1) * HDL].T).astype(bfq),
                "wvT": np.ascontiguousarray(W_qkv[2 * D + g * HDL:2 * D + (g + 1) * HDL].T).astype(bfq),
                "woT": np.ascontiguousarray(woT_full[sl, :]).astype(bfq),
                "mask": mask,
            }
        )

    trace = bool(int(os.environ.get("ATTN_TRACE", "0")))
    last_results = run_bass_kernel_spmd(
        _PROG, in_maps, list(range(8)), trace=trace
    )
    y = np.empty((B, T, D), np.float32)
    for b in range(B):
        yp0 = last_results.results[2 * b]["yT"].astype(np.float32)
        yp1 = last_results.results[2 * b + 1]["yT"].astype(np.float32)
        y[b] = (yp0 + yp1).T
    return y
